# revision 1
# baseline (speedup 1.0000x reference)
"""Trainium2 Bass kernel for nn_CausalSelfAttention_74268574482879.

The reference module's attention scores are overwritten by the causal mask
(q/k are discarded), so softmax weights are uniform over positions <= t:
    y = cummean_T(x) @ W_v @ W_p,   W_v = w_attn[:, 1024:1536]

Distribution: the 4096 rows of (B*T) are split into 8 chunks of 512 rows,
one per NeuronCore.  The only cross-chunk dependency is the column-sum of
all preceding rows in the same batch element; the host passes that tiny
(512,) halo vector per core while slicing the shards.

Per-core dataflow (all matmuls keep operands in natural layout — the PE's
implicit transpose of the stationary operand does all layout work):
  stage A: lhsT=x_tile, rhs=U_scaled  ->  A^T = scale*(cumsum(x))^T  (PSUM)
           + per-tile colsums (lhsT=x_tile, rhs=ones) and a DVE carry add
  stage B: lhsT=Wv,     rhs=A^T       ->  M1^T = (A @ Wv)^T
  stage C: lhsT=M1^T,   rhs=Wp        ->  Y = M1 @ Wp   (natural, DMA out)
"""

import numpy as np

import concourse.bass as bass
import concourse.bacc as bacc
import concourse.mybir as mybir
import concourse.tile as tile
from concourse import bass_utils

N_CORES = 8
B, T, C = 2, 2048, 512
CHUNK = 512               # rows of flattened (B*T) per core
P = 128
NT = CHUNK // P           # 4 row-tiles per chunk
NI = C // P               # 4 col-tiles of the 512 feature dim
F32 = mybir.dt.float32

_STATE = {"nc": None}
TRACE = [False]           # test harness can flip this
LAST_RESULT = [None]      # BassKernelResults of the last run


def _build_nc():
    nc = bacc.Bacc(
        "TRN2", target_bir_lowering=False, debug=False, num_devices=N_CORES
    )
    x_d = nc.dram_tensor("x", (CHUNK, C), F32, kind="ExternalInput")
    wv_d = nc.dram_tensor("wv", (C, C), F32, kind="ExternalInput")
    wp_d = nc.dram_tensor("wp", (C, C), F32, kind="ExternalInput")
    us_d = nc.dram_tensor("us", (P, CHUNK), F32, kind="ExternalInput")
    sc_d = nc.dram_tensor("sc", (P, CHUNK), F32, kind="ExternalInput")
    pc_d = nc.dram_tensor("pc", (P, NI), F32, kind="ExternalInput")
    y_d = nc.dram_tensor("y", (CHUNK, C), F32, kind="ExternalOutput")

    x_ap, wv_ap, wp_ap = x_d.ap(), wv_d.ap(), wp_d.ap()
    us_ap, sc_ap, pc_ap, y_ap = us_d.ap(), sc_d.ap(), pc_d.ap(), y_d.ap()

    with tile.TileContext(nc) as tc:
        with (
            tc.tile_pool(name="io", bufs=1) as io,
            tc.tile_pool(name="tmp", bufs=4) as tmp_pool,
            tc.tile_pool(name="pscol", bufs=1, space="PSUM") as pscol_pool,
            tc.tile_pool(name="psbig", bufs=7, space="PSUM") as ps_pool,
        ):
            # ---- inputs to SBUF ----
            x_sb = []
            for k in range(NT):
                xk = io.tile([P, C], F32, name=f"x{k}")
                nc.sync.dma_start(xk[:], x_ap[k * P : (k + 1) * P, :])
                x_sb.append(xk)
            us_sb = io.tile([P, CHUNK], F32, name="us_sb")
            nc.sync.dma_start(us_sb[:], us_ap[:, :])
            pc_sb = io.tile([P, NI], F32, name="pc_sb")
            nc.sync.dma_start(pc_sb[:], pc_ap[:, :])
            sc_sb = io.tile([P, CHUNK], F32, name="sc_sb")
            nc.sync.dma_start(sc_sb[:], sc_ap[:, :])
            wv_sb = []
            for i in range(NI):
                wvi = io.tile([P, C], F32, name=f"wv{i}")
                nc.sync.dma_start(wvi[:], wv_ap[i * P : (i + 1) * P, :])
                wv_sb.append(wvi)
            wp_sb = []
            for j in range(NI):
                wpj = io.tile([P, C], F32, name=f"wp{j}")
                nc.sync.dma_start(wpj[:], wp_ap[j * P : (j + 1) * P, :])
                wp_sb.append(wpj)

            ones_sb = io.tile([P, 1], F32, name="ones_sb")
            nc.vector.memset(ones_sb[:], 1.0)

            # ---- stage A: per-tile colsums + scaled local cumsum ----
            # pscol[:, i*NT+k] = colsum of x row-tile k, feature slice i
            pscol = pscol_pool.tile([P, NT * NI], F32, name="pscol", tag="pscol")
            for i in range(NI):
                ci = slice(i * P, (i + 1) * P)
                for k in range(NT):
                    col = i * NT + k
                    nc.tensor.matmul(
                        pscol[:, col : col + 1],
                        x_sb[k][:, ci],
                        ones_sb[:],
                        start=True,
                        stop=True,
                    )

            # exclusive running prefixes: Pc[:, i*NT+j] = pc[:,i] + sum_{k<j} colsum_k
            Pc_sb = io.tile([P, NT * NI], F32, name="Pc_sb")
            for i in range(NI):
                nc.vector.tensor_copy(
                    Pc_sb[:, i * NT : i * NT + 1], pc_sb[:, i : i + 1]
                )
                for j in range(1, NT):
                    nc.vector.tensor_add(
                        Pc_sb[:, i * NT + j : i * NT + j + 1],
                        Pc_sb[:, i * NT + j - 1 : i * NT + j],
                        pscol[:, i * NT + j - 1 : i * NT + j],
                    )

            # scaled local cumsum into PSUM:  psA[i][c,
            #   j*P+t] = sum_{s<=t} x[j*P+s, i*P+c] * scale(j*P+t)
            psA = []
            for i in range(NI):
                pai = ps_pool.tile([P, CHUNK], F32, name=f"psA{i}", tag="big")
                ci = slice(i * P, (i + 1) * P)
                for j in range(NT):
                    tj = slice(j * P, (j + 1) * P)
                    nc.tensor.matmul(
                        pai[:, tj], x_sb[j][:, ci], us_sb[:, tj], start=True, stop=True
                    )
                psA.append(pai)

            # A^T = psA + Pc * scale  (carry across row-tiles and chunks)
            A_sb = []
            for i in range(NI):
                ai = io.tile([P, CHUNK], F32, name=f"A{i}")
                for j in range(NT):
                    tj = slice(j * P, (j + 1) * P)
                    col = i * NT + j
                    tmp = tmp_pool.tile([P, P], F32, name="carry", tag="carry")
                    nc.vector.tensor_scalar_mul(
                        tmp[:], sc_sb[:, tj], Pc_sb[:, col : col + 1]
                    )
                    nc.vector.tensor_add(ai[:, tj], psA[i][:, tj], tmp[:])
                A_sb.append(ai)

            # ---- stage B: M1^T = (A @ Wv)^T ----
            M1_sb = []
            for jj in range(NI):
                psm = ps_pool.tile([P, CHUNK], F32, name=f"psM{jj}", tag="big")
                cj = slice(jj * P, (jj + 1) * P)
                for i in range(NI):
                    nc.tensor.matmul(
                        psm[:],
                        wv_sb[i][:, cj],
                        A_sb[i][:],
                        start=(i == 0),
                        stop=(i == NI - 1),
                    )
                m1 = io.tile([P, CHUNK], F32, name=f"M1{jj}")
                nc.vector.tensor_copy(m1[:], psm[:])
                M1_sb.append(m1)

            # ---- stage C: Y = M1 @ Wp  (natural layout) ----
            for tt in range(NT):
                psy = ps_pool.tile([P, C], F32, name=f"psY{tt}", tag="big")
                st = slice(tt * P, (tt + 1) * P)
                for jj in range(NI):
                    nc.tensor.matmul(
                        psy[:],
                        M1_sb[jj][:, st],
                        wp_sb[jj][:],
                        start=(jj == 0),
                        stop=(jj == NI - 1),
                    )
                ysb = io.tile([P, C], F32, name=f"y{tt}")
                nc.vector.tensor_copy(ysb[:], psy[:])
                nc.sync.dma_start(y_ap[st, :], ysb[:])

    nc.compile()
    return nc


def _get_nc():
    if _STATE["nc"] is None:
        _STATE["nc"] = _build_nc()
    return _STATE["nc"]


def _prepare_in_maps(x, w_attn, w_proj):
    x = np.asarray(x, dtype=np.float32)
    w_attn = np.asarray(w_attn, dtype=np.float32)
    w_proj = np.ascontiguousarray(np.asarray(w_proj, dtype=np.float32))
    wv = np.ascontiguousarray(w_attn[:, 2 * C : 3 * C])

    in_maps = []
    for core in range(N_CORES):
        b, tc = divmod(core, T // CHUNK)
        goff = tc * CHUNK
        chunk = np.ascontiguousarray(x[b, goff : goff + CHUNK, :])
        # halo: column-sum of all earlier rows in this batch element
        p = x[b, :goff, :].sum(axis=0, dtype=np.float32) if goff else np.zeros(
            C, np.float32
        )
        pc = np.ascontiguousarray(p.reshape(NI, P).T)  # pc[r, i] = p[i*P + r]
        scale = (1.0 / (goff + np.arange(1, CHUNK + 1))).astype(np.float32)
        sc = np.ascontiguousarray(np.broadcast_to(scale, (P, CHUNK)))
        us = np.zeros((P, CHUNK), np.float32)
        for j in range(NT):
            blk = us[:, j * P : (j + 1) * P]
            tri = np.triu(np.ones((P, P), np.float32))  # s <= t
            blk[:] = tri * scale[j * P : (j + 1) * P][None, :]
        in_maps.append(
            {"x": chunk, "wv": wv, "wp": w_proj, "us": us, "sc": sc, "pc": pc}
        )
    return in_maps


def kernel(x, w_attn, w_proj):
    nc = _get_nc()
    in_maps = _prepare_in_maps(x, w_attn, w_proj)
    res = bass_utils.run_bass_kernel_spmd(
        nc, in_maps, core_ids=list(range(N_CORES)), trace=TRACE[0]
    )
    LAST_RESULT[0] = res
    y = np.empty((B, T, C), np.float32)
    for core in range(N_CORES):
        b, tc = divmod(core, T // CHUNK)
        y[b, tc * CHUNK : (tc + 1) * CHUNK, :] = res.results[core]["y"]
    return y
